# revision 35
# baseline (speedup 1.0000x reference)
"""Self-contained Trainium2 Bass kernel for nn_Attention_37125697306831.

Multi-head attention block: B=4, H=W=48 (N=2304), C=256, 8 heads, head_dim=32,
RoPE (rotate-half), softmax attention, separate Q/K/V projections (K without
bias), output projection with bias.

Sharding: 8 cores = (batch b in 0..3) x (query half in 0..2). Each core
computes Q for its 1152 queries, K/V for all 2304 keys of its batch, and the
attention + output projection for its 1152 query rows. No collectives.

v2 design (per-core):
  - fp16 projections; RoPE combine on DVE (2 STT + add) writes fp16 qT/kT.
  - scores as fp16 matmuls: S.T tiles [128 keys, cw queries], one per
    (head, key tile); PSUM tiles hold a key-tile pair [128, 2, 512].
  - exp split across ScalarE (native Exp activation, ~60%) and DVE
    (Schraudolph bit-trick exp: i16 = s*1477.32*scale + B, bitcast fp16,
    ~2-3% sawtooth error that largely cancels in the softmax ratio).
  - A@V in fp16: lhsT = V block [128 keys, 33] (col 32 = ones so the softmax
    denominator accumulates for free), rhs = exp'd P.T tile, accumulated over
    18 key tiles into av PSUM [128, 4, 512] (2 heads per bank at partition
    offsets 0/64 via tile_position).
  - normalize: DVE reciprocal of the denominator rows, PE K=1 broadcast to
    32 partitions, DVE multiply -> oT fp16 [ci, q].
  - output projection fp16 with bias via a K=1 ones-row matmul; ScalarE
    copies y to SBUF for the fp32 DMA out.

An FP8_SCORES variant (fp8 DoubleRow scores with a q-residual correction in
the second pair slot) is kept behind a flag; it measured 2.0e-2 rel err -
exactly at the harness gate - so fp16 scores ship instead (5e-3, 4x margin).
"""

import numpy as np
from contextlib import ExitStack

import concourse.bass as bass
import concourse.tile as tile
from concourse import bacc, mybir
from concourse.bass_utils import run_bass_kernel_spmd

F32 = mybir.dt.float32
F16 = mybir.dt.float16
F8 = mybir.dt.float8e4
I16 = mybir.dt.int16
AF = mybir.ActivationFunctionType
DR = mybir.MatmulPerfMode.DoubleRow
mul_op = mybir.AluOpType.mult
add_op = mybir.AluOpType.add

B, HH, WW, C = 4, 48, 48, 256
N = HH * WW            # 2304 keys per batch
NQ = N // 2            # 1152 queries per core
NH, HD, D2 = 8, 32, 16
NT = N // 128          # 18 key tiles
ROPE_BASE = 10000.0
SCALE = HD ** -0.5

# Schraudolph fp16 exp constants: i16 = round(1477.3195*x + B); the -44.7
# centers the piecewise-linear 2^frac sawtooth, +0.5 compensates truncation.
SCH_A = 1477.3195
SCH_B = 15360.0 - 44.7 + 0.5

QCH = [(0, 512), (1024, 128), (512, 512)]   # small chunk mid for overlap
KCH = [(0, 512), (512, 512), (1024, 512), (1536, 512), (2048, 256)]

IN_SPECS = [
    ("ctq", [128, NQ], F16), ("stq", [128, NQ], F16),
    ("xT0", [128, N], F16), ("xT1", [128, N], F16),  # x.T ci-halves fp16
    ("xq0", [128, NQ], F16), ("xq1", [128, NQ], F16),  # query-half slices
    ("wq", [128, 2, C], F16), ("wqr", [128, 2, C], F16),  # [:, ci_half, co]
    ("wk", [128, 2, C], F16), ("wkr", [128, 2, C], F16),
    ("wv0", [128, C], F16), ("wv1", [128, C], F16),
    ("wo0", [128, C], F16), ("wo1", [128, C], F16),
    ("qb", [128, 2], F32), ("rqb", [128, 2], F32),   # [:, cg]
    ("vb", [1, C], F16), ("bob", [1, C], F16),
    ("ones", [128, 128], F16),
    ("ct", [128, N], F16), ("st", [128, N], F16),    # unscaled cos/sin rows
]


ROUTE_DVE_ON = True
EXP_SPLIT = "route"  # "route" (per-tile engine routing) or "column"
FP8_SCORES = False  # fp8 scores rejected: k-side fp8 noise ~1.7e-2 rel vs 2e-2 gate


def route_dve(h, tp):
    # ~40% of exp tiles to DVE, interleaved so both engines stay fed.
    return ROUTE_DVE_ON and ((tp * 8 + h) * 2) % 5 < 2


def emit(tc, io, R=1):
    nc = tc.nc
    ctx = ExitStack()
    with ctx:
        consts = ctx.enter_context(tc.tile_pool(name="consts", bufs=1))
        sb = ctx.enter_context(tc.tile_pool(name="sb", bufs=1))
        tmp = ctx.enter_context(tc.tile_pool(name="tmp", bufs=4))
        ptpool = ctx.enter_context(tc.tile_pool(name="pt", bufs=8))
        otpool = ctx.enter_context(tc.tile_pool(name="oT", bufs=2))
        rpool = ctx.enter_context(tc.tile_pool(name="recip", bufs=2))
        scp = ctx.enter_context(tc.tile_pool(name="scp", bufs=6, space="PSUM"))
        avp = ctx.enter_context(tc.tile_pool(name="avp", bufs=1, space="PSUM"))
        # scp: 1-bank [128, 512] f32 tiles; 6 slots + av (2 banks) = 8 banks

        dtypes = {name: dt for name, _, dt in IN_SPECS}

        def load(name):
            shape = next(s for n, s, _ in IN_SPECS if n == name)
            t = consts.tile(shape, dtypes[name], tag=name)
            nc.sync.dma_start(t[:], io[name][:])
            return t

        # load order = first-use order so compute starts ASAP
        wq, wqr = load("wq"), load("wqr")
        xq = [load("xq0"), load("xq1")]
        ctq, stq = load("ctq"), load("stq")
        qb, rqb = load("qb"), load("rqb")
        wk, wkr = load("wk"), load("wkr")
        xT = [load("xT0"), load("xT1")]
        ct, st = load("ct"), load("st")
        wv = [load("wv0"), load("wv1")]
        vb, bob = load("vb"), load("bob")
        ones = load("ones")
        wo = [load("wo0"), load("wo1")]

        # persistent attention operand tiles
        if FP8_SCORES:
            qT8 = [sb.tile([128, 2, NQ], F8, tag=f"qT8{g}", name=f"qT8{g}")
                   for g in range(2)]
            kT8 = [sb.tile([128, 2, N], F8, tag=f"kT8{g}", name=f"kT8{g}")
                   for g in range(2)]
        else:
            qT8 = [sb.tile([128, NQ], F16, tag=f"qT16{g}", name=f"qT16{g}")
                   for g in range(2)]
            kT8 = [sb.tile([128, N], F16, tag=f"kT16{g}", name=f"kT16{g}")
                   for g in range(2)]
        vsb = sb.tile([128, NT * NH * 33], F16, tag="vsb")

        nc.gpsimd.memset(vsb[:], 1.0)

        if R > 1:
            loop_ctx = tc.For_i(0, R, 1)
            loop_ctx.__enter__()

        # ---- phase 1: projections + RoPE ---------------------------------
        def proj_rope(dst, w, wr, bias, rbias, xa, xb, cts, sts, chunks,
                      mode="q"):
            # dst[cg][:, 0, off:off+cw] = (W.T x + b)*cos + (Wr.T x + rb)*sin
            for off, cw in chunks:
                for cg in range(2):
                    ps0 = scp.tile([128, 512], F32, tag="sc")
                    nc.tensor.matmul(ps0[:, :cw],
                                     w[:, 0, bass.ts(cg, 128)],
                                     xa[:, off:off + cw],
                                     start=True, stop=False)
                    nc.tensor.matmul(ps0[:, :cw],
                                     w[:, 1, bass.ts(cg, 128)],
                                     xb[:, off:off + cw],
                                     start=False, stop=True)
                    ps1 = scp.tile([128, 512], F32, tag="sc")
                    nc.tensor.matmul(ps1[:, :cw],
                                     wr[:, 0, bass.ts(cg, 128)],
                                     xa[:, off:off + cw],
                                     start=True, stop=False)
                    nc.tensor.matmul(ps1[:, :cw],
                                     wr[:, 1, bass.ts(cg, 128)],
                                     xb[:, off:off + cw],
                                     start=False, stop=True)
                    t1 = tmp.tile([128, 512], F16, tag="t1")
                    b0 = bias[:, cg:cg + 1] if bias is not None else 0.0
                    b1 = rbias[:, cg:cg + 1] if rbias is not None else 0.0
                    nc.vector.scalar_tensor_tensor(
                        t1[:, 0:cw], ps0[:, 0:cw], b0,
                        cts[:, off:off + cw], op0=add_op, op1=mul_op)
                    t2 = tmp.tile([128, 512], F16, tag="t2")
                    nc.vector.scalar_tensor_tensor(
                        t2[:, 0:cw], ps1[:, 0:cw], b1,
                        sts[:, off:off + cw], op0=add_op, op1=mul_op)
                    if not FP8_SCORES:
                        nc.vector.tensor_add(dst[cg][:, off:off + cw],
                                             t1[:, 0:cw], t2[:, 0:cw])
                    elif mode == "k":
                        # slot0 = k8, slot1 = k8/16 (pairs with 16*dq)
                        nc.vector.tensor_add(dst[cg][:, 0, off:off + cw],
                                             t1[:, 0:cw], t2[:, 0:cw])
                        nc.scalar.mul(dst[cg][:, 1, off:off + cw],
                                      dst[cg][:, 0, off:off + cw], 1.0 / 16.0)
                    else:
                        # ct/st pre-scaled x16: q16s = 16*q_rope;
                        # slot0 = q8 = q16s/16; slot1 = q16s - 16*q8 = 16*dq
                        q16s = tmp.tile([128, 512], F16, tag="q16s")
                        nc.vector.tensor_add(q16s[:, 0:cw],
                                             t1[:, 0:cw], t2[:, 0:cw])
                        nc.scalar.mul(dst[cg][:, 0, off:off + cw],
                                      q16s[:, 0:cw], 1.0 / 16.0)
                        nc.vector.scalar_tensor_tensor(
                            dst[cg][:, 1, off:off + cw],
                            dst[cg][:, 0, off:off + cw], -16.0,
                            q16s[:, 0:cw], op0=mul_op, op1=add_op)

        proj_rope(qT8, wq, wqr, qb, rqb, xq[0], xq[1], ctq, stq, QCH,
                  mode="q")
        proj_rope(kT8, wk, wkr, None, None, xT[0], xT[1], ct, st, KCH,
                  mode="k")

        # V projection fp16 + bias via K=1 ones matmul; copy into 33-wide
        # blocks (33rd col stays 1.0 from the memset) on ScalarE.
        for t in range(NT):
            ps = scp.tile([128, 512], F32, tag="sc")
            nc.tensor.matmul(ps[:, :C], xT[0][:, bass.ts(t, 128)], wv[0][:],
                             start=True, stop=False)
            nc.tensor.matmul(ps[:, :C], xT[1][:, bass.ts(t, 128)], wv[1][:],
                             start=False, stop=False)
            nc.tensor.matmul(ps[:, :C], ones[0:1, :], vb[:],
                             start=False, stop=True)
            vdst = vsb[:, t * NH * 33:(t + 1) * NH * 33]
            vdst = vdst.rearrange("p (h c) -> p h c", c=33)
            psrc = ps[:, 0:C].rearrange("p (h c) -> p h c", c=32)
            nc.scalar.copy(vdst[:, :, 0:32], psrc[:])


        if "dbg_q" in io:
            for g in range(2):
                qs = qT8[g][:, 0, :] if FP8_SCORES else qT8[g][:]
                ks = kT8[g][:, 0, :] if FP8_SCORES else kT8[g][:]
                nc.sync.dma_start(io["dbg_q"][g], qs)
                nc.sync.dma_start(io["dbg_k"][g], ks)

        # ---- phase 2: attention + output projection ----------------------
        # Head halves (hh) so av needs only 2 banks and double-buffers:
        # the normalize chain of one half overlaps the next half's matmuls.
        # Each tile's exp is column-split across ScalarE (native Exp) and
        # DVE (Schraudolph) so both engines work the same tile in parallel.
        for off, cw in QCH:
            c1 = (cw * 37) // 64 // 4 * 4       # ~58% of columns to ScalarE
            oT = [otpool.tile([128, 512], F16, tag=f"o{k}", name=f"o{k}")
                  for k in range(2)]
            for hh in range(2):
                av = avp.tile([128, 2, 512], F32, tag="av")
                for tp in range(NT // 2):
                    for hl in range(4):
                        h = 4 * hh + hl
                        cg, h4 = h // 4, h % 4
                        g, j = hl // 2, hl % 2
                        for i in range(2):
                            t = 2 * tp + i
                            sc = scp.tile([128, 512], F32, tag="sc")
                            if FP8_SCORES:
                                nc.tensor.matmul(
                                    sc[:, :cw],
                                    kT8[cg][bass.ts(h4, 32), :,
                                            bass.ts(t, 128)],
                                    qT8[cg][bass.ts(h4, 32), :, off:off + cw],
                                    start=True, stop=True, perf_mode=DR,
                                    tile_position=(32 * h4, 0))
                            else:
                                nc.tensor.matmul(
                                    sc[:, :cw],
                                    kT8[cg][bass.ts(h4, 32), bass.ts(t, 128)],
                                    qT8[cg][bass.ts(h4, 32), off:off + cw],
                                    start=True, stop=True,
                                    tile_position=(32 * h4, 0))
                            pt = ptpool.tile([128, 512], F16, tag="pt")
                            if (((tp * 8 + h) * 2 + i) * 2) % 5 < 2:
                                nc.vector.tensor_scalar(
                                    pt.bitcast(I16)[:, 0:cw], sc[:, 0:cw],
                                    SCH_A * SCALE, SCH_B, mul_op, add_op)
                            else:
                                nc.scalar.activation(pt[:, 0:cw], sc[:, 0:cw],
                                                     AF.Exp, scale=SCALE)
                            nc.tensor.matmul(
                                av[64 * j:64 * j + 33, g, 0:cw],
                                vsb[:, (t * NH + h) * 33:
                                       (t * NH + h + 1) * 33],
                                pt[:, 0:cw],
                                start=(t == 0), stop=(t == NT - 1),
                                tile_position=(0, 64 * j),
                                skip_group_check=True)

                # normalize this head half: reciprocal of denominator rows,
                # K=1 broadcast to 32 partitions, multiply into oT fp16.
                rsb = rpool.tile([128, 2, 512], F16, tag="rs")
                with nc.allow_low_precision("fp16 softmax scale rows"):
                    for j in range(2):
                        r = 64 * j + 32
                        nc.vector.reciprocal(rsb[r:r + 1, :, 0:cw],
                                             av[r:r + 1, :, 0:cw])
                rfsb = rpool.tile([128, 2, 512], F16, tag="rfsb")
                for g in range(2):
                    rf = scp.tile([128, 512], F32, tag="sc")
                    for j in range(2):
                        r = 64 * j + 32
                        nc.tensor.matmul(
                            rf[64 * j:64 * j + 32, 0:cw],
                            ones[r:r + 1, 0:32], rsb[r:r + 1, g, 0:cw],
                            start=True, stop=True,
                            tile_position=(r, 64 * j), skip_group_check=True)
                    nc.scalar.copy(rfsb[:, g, 0:cw], rf[:, 0:cw])
                for hl in range(4):
                    g, j = hl // 2, hl % 2
                    nc.vector.tensor_mul(
                        oT[hh][32 * hl:32 * hl + 32, 0:cw],
                        av[64 * j:64 * j + 32, g, 0:cw],
                        rfsb[64 * j:64 * j + 32, g, 0:cw])

            # output projection + bias, y DMA straight from PSUM
            for s in range(cw // 128):
                yps = scp.tile([128, 512], F32, tag="sc")
                nc.tensor.matmul(yps[:, :C], oT[0][:, bass.ts(s, 128)],
                                 wo[0][:], start=True, stop=False)
                nc.tensor.matmul(yps[:, :C], oT[1][:, bass.ts(s, 128)],
                                 wo[1][:], start=False, stop=False)
                nc.tensor.matmul(yps[:, :C], ones[0:1, 0:128], bob[:],
                                 start=False, stop=True)
                ysb = tmp.tile([128, 512], F32, tag="ysb")
                nc.scalar.copy(ysb[:, 0:C], yps[:, 0:C])
                nc.sync.dma_start(
                    io["y"][off + 128 * s: off + 128 * (s + 1), :],
                    ysb[:, 0:C])

        if R > 1:
            loop_ctx.__exit__(None, None, None)


def build_nc(R=1):
    nc = bacc.Bacc("TRN2", target_bir_lowering=False, debug=False,
                   enable_asserts=True, num_devices=8)
    io = {}
    for name, shape, dt in IN_SPECS:
        io[name] = nc.dram_tensor(name, shape, dt, kind="ExternalInput").ap()
    io["y"] = nc.dram_tensor("y", [NQ, C], F32, kind="ExternalOutput").ap()

    with tile.TileContext(nc) as tc:
        emit(tc, io, R=R)
    nc.compile()
    return nc


def host_inputs(x, Wq, q_bias, Wk, Wv, v_bias, Wo, bo):
    """Per-core input maps (host-side sharding + layout prep)."""
    import ml_dtypes
    f8d = ml_dtypes.float8_e4m3fn

    xf = np.ascontiguousarray(x.reshape(B, N, C))

    inv_freq = 1.0 / (ROPE_BASE ** (np.arange(0, HD, 2, dtype=np.float64) / HD))
    pos = np.arange(N, dtype=np.float64)
    ang = pos[:, None] * inv_freq[None, :]          # [N, 16]
    cos_t, sin_t = np.cos(ang), np.sin(ang)
    f = np.arange(128) % D2                         # row r -> freq index
    CT = cos_t[:, f].T                              # [128, N]
    ST = sin_t[:, f].T

    # signed rotate-half permutation RM [C, C]: partner = RM @ q
    RM = np.zeros((C, C), dtype=np.float64)
    for p in range(C):
        j = p % HD
        if j < D2:
            RM[p, p + D2] = -1.0
        else:
            RM[p, p - D2] = 1.0

    Wq64, Wk64 = Wq.astype(np.float64), Wk.astype(np.float64)
    Wqr = RM @ Wq64
    Wkr = RM @ Wk64
    rqb = RM @ q_bias.astype(np.float64)

    f16 = lambda a: np.ascontiguousarray(a, dtype=np.float16)
    f32 = lambda a: np.ascontiguousarray(a, dtype=np.float32)

    def wpack(WT):  # WT [C, C] -> [128, 2(ci half), C] fp16
        return f16(WT.reshape(2, 128, C).transpose(1, 0, 2))

    common = {
        "wq": wpack(Wq64.T), "wqr": wpack(Wqr.T),
        "wk": wpack(Wk64.T), "wkr": wpack(Wkr.T),
        "wv0": f16(Wv.T[0:128]), "wv1": f16(Wv.T[128:256]),
        "wo0": f16(Wo.T[0:128]), "wo1": f16(Wo.T[128:256]),
        "qb": f32(q_bias.reshape(2, 128).T),
        "rqb": f32(rqb.reshape(2, 128).T),
        "vb": f16(v_bias[None, :]), "bob": f16(bo[None, :]),
        "ones": np.ones((128, 128), dtype=np.float16),
        "ct": f16(CT), "st": f16(ST),
    }
    qsc = 16.0 if FP8_SCORES else 1.0   # q-residual trick pre-scale
    in_maps = []
    for core in range(8):
        b, qhalf = core // 2, core % 2
        qoff = qhalf * NQ
        xT = xf[b].T                                # [C, N]
        m = dict(common)
        m["ctq"] = f16(CT[:, qoff:qoff + NQ] * qsc)
        m["stq"] = f16(ST[:, qoff:qoff + NQ] * qsc)
        m["xT0"] = f16(xT[0:128])
        m["xT1"] = f16(xT[128:256])
        m["xq0"] = f16(xT[0:128, qoff:qoff + NQ])
        m["xq1"] = f16(xT[128:256, qoff:qoff + NQ])
        in_maps.append(m)
    return in_maps


_NC_CACHE = {}


def get_nc(R=1):
    if R not in _NC_CACHE:
        _NC_CACHE[R] = build_nc(R)
    return _NC_CACHE[R]


def kernel(**inputs):
    inputs = {k: np.asarray(v, dtype=np.float32) for k, v in inputs.items()}
    in_maps = host_inputs(**inputs)
    nc = get_nc()
    res = run_bass_kernel_spmd(nc, in_maps, core_ids=list(range(8)))
    out = np.empty((B, N, C), dtype=np.float32)
    for core in range(8):
        b, qhalf = core // 2, core % 2
        qoff = qhalf * NQ
        out[b, qoff:qoff + NQ, :] = res.results[core]["y"]
    return out.reshape(B, HH, WW, C)


# revision 36
# speedup vs baseline: 1.1223x; 1.1223x over previous
"""Self-contained Trainium2 Bass kernel for nn_Attention_37125697306831.

Multi-head attention block: B=4, H=W=48 (N=2304), C=256, 8 heads, head_dim=32,
RoPE (rotate-half), softmax attention, separate Q/K/V projections (K without
bias), output projection with bias.

Sharding: 8 cores = (batch b in 0..3) x (query half in 0..2). Each core
computes Q for its 1152 queries, K/V for all 2304 keys of its batch, and the
attention + output projection for its 1152 query rows. No collectives.

v2 design (per-core):
  - fp16 projections; RoPE combine on DVE (2 STT + add) writes fp16 qT/kT.
  - scores as fp16 matmuls: S.T tiles [128 keys, cw queries], one per
    (head, key tile); PSUM tiles hold a key-tile pair [128, 2, 512].
  - exp split across ScalarE (native Exp activation, ~60%) and DVE
    (Schraudolph bit-trick exp: i16 = s*1477.32*scale + B, bitcast fp16,
    ~2-3% sawtooth error that largely cancels in the softmax ratio).
  - A@V in fp16: lhsT = V block [128 keys, 33] (col 32 = ones so the softmax
    denominator accumulates for free), rhs = exp'd P.T tile, accumulated over
    18 key tiles into av PSUM [128, 4, 512] (2 heads per bank at partition
    offsets 0/64 via tile_position).
  - normalize: DVE reciprocal of the denominator rows, PE K=1 broadcast to
    32 partitions, DVE multiply -> oT fp16 [ci, q].
  - output projection fp16 with bias via a K=1 ones-row matmul; ScalarE
    copies y to SBUF for the fp32 DMA out.

An FP8_SCORES variant (fp8 DoubleRow scores with a q-residual correction in
the second pair slot) is kept behind a flag; it measured 2.0e-2 rel err -
exactly at the harness gate - so fp16 scores ship instead (5e-3, 4x margin).
"""

import numpy as np
from contextlib import ExitStack

import concourse.bass as bass
import concourse.tile as tile
from concourse import bacc, mybir
from concourse.bass_utils import run_bass_kernel_spmd

F32 = mybir.dt.float32
F16 = mybir.dt.float16
F8 = mybir.dt.float8e4
I16 = mybir.dt.int16
AF = mybir.ActivationFunctionType
DR = mybir.MatmulPerfMode.DoubleRow
mul_op = mybir.AluOpType.mult
add_op = mybir.AluOpType.add

B, HH, WW, C = 4, 48, 48, 256
N = HH * WW            # 2304 keys per batch
NQ = N // 2            # 1152 queries per core
NH, HD, D2 = 8, 32, 16
NT = N // 128          # 18 key tiles
ROPE_BASE = 10000.0
SCALE = HD ** -0.5

# Schraudolph fp16 exp constants: i16 = round(1477.3195*x + B); the -44.7
# centers the piecewise-linear 2^frac sawtooth, +0.5 compensates truncation.
SCH_A = 1477.3195
SCH_B = 15360.0 - 44.7 + 0.5

QCH = [(0, 512), (1024, 128), (512, 512)]   # small chunk mid for overlap
KCH = [(0, 512), (512, 512), (1024, 512), (1536, 512), (2048, 256)]

IN_SPECS = [
    ("ctq", [128, NQ], F16), ("stq", [128, NQ], F16),
    ("xT0", [128, N], F16), ("xT1", [128, N], F16),  # x.T ci-halves fp16
    ("xq0", [128, NQ], F16), ("xq1", [128, NQ], F16),  # query-half slices
    ("wq", [128, 2, C], F16), ("wqr", [128, 2, C], F16),  # [:, ci_half, co]
    ("wk", [128, 2, C], F16), ("wkr", [128, 2, C], F16),
    ("wv0", [128, C], F16), ("wv1", [128, C], F16),
    ("wo0", [128, C], F16), ("wo1", [128, C], F16),
    ("qb", [128, 2], F32), ("rqb", [128, 2], F32),   # [:, cg]
    ("vb", [1, C], F16), ("bob", [1, C], F16),
    ("ones", [128, 128], F16),
    ("ct", [128, N], F16), ("st", [128, N], F16),    # unscaled cos/sin rows
]


ROUTE_DVE_ON = True
EXP_SPLIT = "route"  # "route" (per-tile engine routing) or "column"
FP8_SCORES = False  # fp8 scores rejected: k-side fp8 noise ~1.7e-2 rel vs 2e-2 gate


def route_dve(h, tp):
    # ~40% of exp tiles to DVE, interleaved so both engines stay fed.
    return ROUTE_DVE_ON and ((tp * 8 + h) * 2) % 5 < 2


def emit(tc, io, R=1):
    nc = tc.nc
    ctx = ExitStack()
    with ctx:
        consts = ctx.enter_context(tc.tile_pool(name="consts", bufs=1))
        sb = ctx.enter_context(tc.tile_pool(name="sb", bufs=1))
        tmp = ctx.enter_context(tc.tile_pool(name="tmp", bufs=4))
        ptpool = ctx.enter_context(tc.tile_pool(name="pt", bufs=8))
        otpool = ctx.enter_context(tc.tile_pool(name="oT", bufs=2))
        rpool = ctx.enter_context(tc.tile_pool(name="recip", bufs=2))
        scp = ctx.enter_context(tc.tile_pool(name="scp", bufs=6, space="PSUM"))
        avp = ctx.enter_context(tc.tile_pool(name="avp", bufs=1, space="PSUM"))
        # scp: 1-bank [128, 512] f32 tiles; 6 slots + av (2 banks) = 8 banks

        dtypes = {name: dt for name, _, dt in IN_SPECS}

        def load(name):
            shape = next(s for n, s, _ in IN_SPECS if n == name)
            t = consts.tile(shape, dtypes[name], tag=name)
            nc.sync.dma_start(t[:], io[name][:])
            return t

        # load order = first-use order so compute starts ASAP
        wq, wqr = load("wq"), load("wqr")
        xq = [load("xq0"), load("xq1")]
        ctq, stq = load("ctq"), load("stq")
        qb, rqb = load("qb"), load("rqb")
        wk, wkr = load("wk"), load("wkr")
        xT = [load("xT0"), load("xT1")]
        ct, st = load("ct"), load("st")
        wv = [load("wv0"), load("wv1")]
        vb, bob = load("vb"), load("bob")
        ones = load("ones")
        wo = [load("wo0"), load("wo1")]

        # persistent attention operand tiles
        if FP8_SCORES:
            qT8 = [sb.tile([128, 2, NQ], F8, tag=f"qT8{g}", name=f"qT8{g}")
                   for g in range(2)]
            kT8 = [sb.tile([128, 2, N], F8, tag=f"kT8{g}", name=f"kT8{g}")
                   for g in range(2)]
        else:
            qT8 = [sb.tile([128, NQ], F16, tag=f"qT16{g}", name=f"qT16{g}")
                   for g in range(2)]
            kT8 = [sb.tile([128, N], F16, tag=f"kT16{g}", name=f"kT16{g}")
                   for g in range(2)]
        vsb = sb.tile([128, NT * NH * 33], F16, tag="vsb")

        nc.gpsimd.memset(vsb[:], 1.0)

        if R > 1:
            loop_ctx = tc.For_i(0, R, 1)
            loop_ctx.__enter__()

        # ---- phase 1: projections + RoPE ---------------------------------
        def proj_rope(dst, w, wr, bias, rbias, xa, xb, cts, sts, chunks,
                      mode="q"):
            # dst[cg][:, 0, off:off+cw] = (W.T x + b)*cos + (Wr.T x + rb)*sin
            for off, cw in chunks:
                for cg in range(2):
                    ps0 = scp.tile([128, 512], F32, tag="sc")
                    nc.tensor.matmul(ps0[:, :cw],
                                     w[:, 0, bass.ts(cg, 128)],
                                     xa[:, off:off + cw],
                                     start=True, stop=False)
                    nc.tensor.matmul(ps0[:, :cw],
                                     w[:, 1, bass.ts(cg, 128)],
                                     xb[:, off:off + cw],
                                     start=False, stop=True)
                    ps1 = scp.tile([128, 512], F32, tag="sc")
                    nc.tensor.matmul(ps1[:, :cw],
                                     wr[:, 0, bass.ts(cg, 128)],
                                     xa[:, off:off + cw],
                                     start=True, stop=False)
                    nc.tensor.matmul(ps1[:, :cw],
                                     wr[:, 1, bass.ts(cg, 128)],
                                     xb[:, off:off + cw],
                                     start=False, stop=True)
                    t1 = tmp.tile([128, 512], F16, tag="t1")
                    b0 = bias[:, cg:cg + 1] if bias is not None else 0.0
                    b1 = rbias[:, cg:cg + 1] if rbias is not None else 0.0
                    nc.vector.scalar_tensor_tensor(
                        t1[:, 0:cw], ps0[:, 0:cw], b0,
                        cts[:, off:off + cw], op0=add_op, op1=mul_op)
                    t2 = tmp.tile([128, 512], F16, tag="t2")
                    nc.vector.scalar_tensor_tensor(
                        t2[:, 0:cw], ps1[:, 0:cw], b1,
                        sts[:, off:off + cw], op0=add_op, op1=mul_op)
                    if not FP8_SCORES:
                        # SBUF-only add runs on the otherwise-idle Pool
                        # engine, shortening the DVE projection chain.
                        nc.gpsimd.tensor_add(dst[cg][:, off:off + cw],
                                             t1[:, 0:cw], t2[:, 0:cw])
                    elif mode == "k":
                        # slot0 = k8, slot1 = k8/16 (pairs with 16*dq)
                        nc.vector.tensor_add(dst[cg][:, 0, off:off + cw],
                                             t1[:, 0:cw], t2[:, 0:cw])
                        nc.scalar.mul(dst[cg][:, 1, off:off + cw],
                                      dst[cg][:, 0, off:off + cw], 1.0 / 16.0)
                    else:
                        # ct/st pre-scaled x16: q16s = 16*q_rope;
                        # slot0 = q8 = q16s/16; slot1 = q16s - 16*q8 = 16*dq
                        q16s = tmp.tile([128, 512], F16, tag="q16s")
                        nc.vector.tensor_add(q16s[:, 0:cw],
                                             t1[:, 0:cw], t2[:, 0:cw])
                        nc.scalar.mul(dst[cg][:, 0, off:off + cw],
                                      q16s[:, 0:cw], 1.0 / 16.0)
                        nc.vector.scalar_tensor_tensor(
                            dst[cg][:, 1, off:off + cw],
                            dst[cg][:, 0, off:off + cw], -16.0,
                            q16s[:, 0:cw], op0=mul_op, op1=add_op)

        proj_rope(qT8, wq, wqr, qb, rqb, xq[0], xq[1], ctq, stq, QCH,
                  mode="q")
        proj_rope(kT8, wk, wkr, None, None, xT[0], xT[1], ct, st, KCH,
                  mode="k")

        # V projection fp16 + bias via K=1 ones matmul; copy into 33-wide
        # blocks (33rd col stays 1.0 from the memset) on ScalarE.
        for t in range(NT):
            ps = scp.tile([128, 512], F32, tag="sc")
            nc.tensor.matmul(ps[:, :C], xT[0][:, bass.ts(t, 128)], wv[0][:],
                             start=True, stop=False)
            nc.tensor.matmul(ps[:, :C], xT[1][:, bass.ts(t, 128)], wv[1][:],
                             start=False, stop=False)
            nc.tensor.matmul(ps[:, :C], ones[0:1, :], vb[:],
                             start=False, stop=True)
            vdst = vsb[:, t * NH * 33:(t + 1) * NH * 33]
            vdst = vdst.rearrange("p (h c) -> p h c", c=33)
            psrc = ps[:, 0:C].rearrange("p (h c) -> p h c", c=32)
            nc.scalar.copy(vdst[:, :, 0:32], psrc[:])


        if "dbg_q" in io:
            for g in range(2):
                qs = qT8[g][:, 0, :] if FP8_SCORES else qT8[g][:]
                ks = kT8[g][:, 0, :] if FP8_SCORES else kT8[g][:]
                nc.sync.dma_start(io["dbg_q"][g], qs)
                nc.sync.dma_start(io["dbg_k"][g], ks)

        # ---- phase 2: attention + output projection ----------------------
        # Head halves (hh) so av needs only 2 banks and double-buffers:
        # the normalize chain of one half overlaps the next half's matmuls.
        # Each tile's exp is column-split across ScalarE (native Exp) and
        # DVE (Schraudolph) so both engines work the same tile in parallel.
        for off, cw in QCH:
            c1 = (cw * 37) // 64 // 4 * 4       # ~58% of columns to ScalarE
            oT = [otpool.tile([128, 512], F16, tag=f"o{k}", name=f"o{k}")
                  for k in range(2)]
            for hh in range(2):
                av = avp.tile([128, 2, 512], F32, tag="av")
                for tp in range(NT // 2):
                    for hl in range(4):
                        h = 4 * hh + hl
                        cg, h4 = h // 4, h % 4
                        g, j = hl // 2, hl % 2
                        for i in range(2):
                            t = 2 * tp + i
                            sc = scp.tile([128, 512], F32, tag="sc")
                            if FP8_SCORES:
                                nc.tensor.matmul(
                                    sc[:, :cw],
                                    kT8[cg][bass.ts(h4, 32), :,
                                            bass.ts(t, 128)],
                                    qT8[cg][bass.ts(h4, 32), :, off:off + cw],
                                    start=True, stop=True, perf_mode=DR,
                                    tile_position=(32 * h4, 0))
                            else:
                                nc.tensor.matmul(
                                    sc[:, :cw],
                                    kT8[cg][bass.ts(h4, 32), bass.ts(t, 128)],
                                    qT8[cg][bass.ts(h4, 32), off:off + cw],
                                    start=True, stop=True,
                                    tile_position=(32 * h4, 0))
                            pt = ptpool.tile([128, 512], F16, tag="pt")
                            if (((tp * 8 + h) * 2 + i) * 2) % 5 < 2:
                                nc.vector.tensor_scalar(
                                    pt.bitcast(I16)[:, 0:cw], sc[:, 0:cw],
                                    SCH_A * SCALE, SCH_B, mul_op, add_op)
                            else:
                                nc.scalar.activation(pt[:, 0:cw], sc[:, 0:cw],
                                                     AF.Exp, scale=SCALE)
                            nc.tensor.matmul(
                                av[64 * j:64 * j + 33, g, 0:cw],
                                vsb[:, (t * NH + h) * 33:
                                       (t * NH + h + 1) * 33],
                                pt[:, 0:cw],
                                start=(t == 0), stop=(t == NT - 1),
                                tile_position=(0, 64 * j),
                                skip_group_check=True)

                # normalize this head half: reciprocal of denominator rows,
                # K=1 broadcast to 32 partitions, multiply into oT fp16.
                rsb = rpool.tile([128, 2, 512], F16, tag="rs")
                with nc.allow_low_precision("fp16 softmax scale rows"):
                    for j in range(2):
                        r = 64 * j + 32
                        nc.vector.reciprocal(rsb[r:r + 1, :, 0:cw],
                                             av[r:r + 1, :, 0:cw])
                rfsb = rpool.tile([128, 2, 512], F16, tag="rfsb")
                for g in range(2):
                    rf = scp.tile([128, 512], F32, tag="sc")
                    for j in range(2):
                        r = 64 * j + 32
                        nc.tensor.matmul(
                            rf[64 * j:64 * j + 32, 0:cw],
                            ones[r:r + 1, 0:32], rsb[r:r + 1, g, 0:cw],
                            start=True, stop=True,
                            tile_position=(r, 64 * j), skip_group_check=True)
                    nc.scalar.copy(rfsb[:, g, 0:cw], rf[:, 0:cw])
                for hl in range(4):
                    g, j = hl // 2, hl % 2
                    nc.vector.tensor_mul(
                        oT[hh][32 * hl:32 * hl + 32, 0:cw],
                        av[64 * j:64 * j + 32, g, 0:cw],
                        rfsb[64 * j:64 * j + 32, g, 0:cw])

            # output projection + bias, y DMA straight from PSUM
            for s in range(cw // 128):
                yps = scp.tile([128, 512], F32, tag="sc")
                nc.tensor.matmul(yps[:, :C], oT[0][:, bass.ts(s, 128)],
                                 wo[0][:], start=True, stop=False)
                nc.tensor.matmul(yps[:, :C], oT[1][:, bass.ts(s, 128)],
                                 wo[1][:], start=False, stop=False)
                nc.tensor.matmul(yps[:, :C], ones[0:1, 0:128], bob[:],
                                 start=False, stop=True)
                ysb = tmp.tile([128, 512], F32, tag="ysb")
                nc.scalar.copy(ysb[:, 0:C], yps[:, 0:C])
                nc.sync.dma_start(
                    io["y"][off + 128 * s: off + 128 * (s + 1), :],
                    ysb[:, 0:C])

        if R > 1:
            loop_ctx.__exit__(None, None, None)


def build_nc(R=1):
    nc = bacc.Bacc("TRN2", target_bir_lowering=False, debug=False,
                   enable_asserts=True, num_devices=8)
    io = {}
    for name, shape, dt in IN_SPECS:
        io[name] = nc.dram_tensor(name, shape, dt, kind="ExternalInput").ap()
    io["y"] = nc.dram_tensor("y", [NQ, C], F32, kind="ExternalOutput").ap()

    with tile.TileContext(nc) as tc:
        emit(tc, io, R=R)
    nc.compile()
    return nc


def host_inputs(x, Wq, q_bias, Wk, Wv, v_bias, Wo, bo):
    """Per-core input maps (host-side sharding + layout prep)."""
    import ml_dtypes
    f8d = ml_dtypes.float8_e4m3fn

    xf = np.ascontiguousarray(x.reshape(B, N, C))

    inv_freq = 1.0 / (ROPE_BASE ** (np.arange(0, HD, 2, dtype=np.float64) / HD))
    pos = np.arange(N, dtype=np.float64)
    ang = pos[:, None] * inv_freq[None, :]          # [N, 16]
    cos_t, sin_t = np.cos(ang), np.sin(ang)
    f = np.arange(128) % D2                         # row r -> freq index
    CT = cos_t[:, f].T                              # [128, N]
    ST = sin_t[:, f].T

    # signed rotate-half permutation RM [C, C]: partner = RM @ q
    RM = np.zeros((C, C), dtype=np.float64)
    for p in range(C):
        j = p % HD
        if j < D2:
            RM[p, p + D2] = -1.0
        else:
            RM[p, p - D2] = 1.0

    Wq64, Wk64 = Wq.astype(np.float64), Wk.astype(np.float64)
    Wqr = RM @ Wq64
    Wkr = RM @ Wk64
    rqb = RM @ q_bias.astype(np.float64)

    f16 = lambda a: np.ascontiguousarray(a, dtype=np.float16)
    f32 = lambda a: np.ascontiguousarray(a, dtype=np.float32)

    def wpack(WT):  # WT [C, C] -> [128, 2(ci half), C] fp16
        return f16(WT.reshape(2, 128, C).transpose(1, 0, 2))

    common = {
        "wq": wpack(Wq64.T), "wqr": wpack(Wqr.T),
        "wk": wpack(Wk64.T), "wkr": wpack(Wkr.T),
        "wv0": f16(Wv.T[0:128]), "wv1": f16(Wv.T[128:256]),
        "wo0": f16(Wo.T[0:128]), "wo1": f16(Wo.T[128:256]),
        "qb": f32(q_bias.reshape(2, 128).T),
        "rqb": f32(rqb.reshape(2, 128).T),
        "vb": f16(v_bias[None, :]), "bob": f16(bo[None, :]),
        "ones": np.ones((128, 128), dtype=np.float16),
        "ct": f16(CT), "st": f16(ST),
    }
    qsc = 16.0 if FP8_SCORES else 1.0   # q-residual trick pre-scale
    in_maps = []
    for core in range(8):
        b, qhalf = core // 2, core % 2
        qoff = qhalf * NQ
        xT = xf[b].T                                # [C, N]
        m = dict(common)
        m["ctq"] = f16(CT[:, qoff:qoff + NQ] * qsc)
        m["stq"] = f16(ST[:, qoff:qoff + NQ] * qsc)
        m["xT0"] = f16(xT[0:128])
        m["xT1"] = f16(xT[128:256])
        m["xq0"] = f16(xT[0:128, qoff:qoff + NQ])
        m["xq1"] = f16(xT[128:256, qoff:qoff + NQ])
        in_maps.append(m)
    return in_maps


_NC_CACHE = {}


def get_nc(R=1):
    if R not in _NC_CACHE:
        _NC_CACHE[R] = build_nc(R)
    return _NC_CACHE[R]


def kernel(**inputs):
    inputs = {k: np.asarray(v, dtype=np.float32) for k, v in inputs.items()}
    in_maps = host_inputs(**inputs)
    nc = get_nc()
    res = run_bass_kernel_spmd(nc, in_maps, core_ids=list(range(8)))
    out = np.empty((B, N, C), dtype=np.float32)
    for core in range(8):
        b, qhalf = core // 2, core % 2
        qoff = qhalf * NQ
        out[b, qoff:qoff + NQ, :] = res.results[core]["y"]
    return out.reshape(B, HH, WW, C)


# revision 38
# speedup vs baseline: 1.2668x; 1.1287x over previous
"""Self-contained Trainium2 Bass kernel for nn_Attention_37125697306831.

Multi-head attention block: B=4, H=W=48 (N=2304), C=256, 8 heads, head_dim=32,
RoPE (rotate-half), softmax attention, separate Q/K/V projections (K without
bias), output projection with bias.

Sharding: 8 cores = (batch b in 0..3) x (query half in 0..2). Each core
computes Q for its 1152 queries, K/V for all 2304 keys of its batch, and the
attention + output projection for its 1152 query rows. No collectives.

v2 design (per-core):
  - fp16 projections; RoPE combine on DVE (2 STT + add) writes fp16 qT/kT.
  - scores as fp16 matmuls: S.T tiles [128 keys, cw queries], one per
    (head, key tile); PSUM tiles hold a key-tile pair [128, 2, 512].
  - exp split across ScalarE (native Exp activation, ~60%) and DVE
    (Schraudolph bit-trick exp: i16 = s*1477.32*scale + B, bitcast fp16,
    ~2-3% sawtooth error that largely cancels in the softmax ratio).
  - A@V in fp16: lhsT = V block [128 keys, 33] (col 32 = ones so the softmax
    denominator accumulates for free), rhs = exp'd P.T tile, accumulated over
    18 key tiles into av PSUM [128, 4, 512] (2 heads per bank at partition
    offsets 0/64 via tile_position).
  - normalize: DVE reciprocal of the denominator rows, PE K=1 broadcast to
    32 partitions, DVE multiply -> oT fp16 [ci, q].
  - output projection fp16 with bias via a K=1 ones-row matmul; ScalarE
    copies y to SBUF for the fp32 DMA out.

An FP8_SCORES variant (fp8 DoubleRow scores with a q-residual correction in
the second pair slot) is kept behind a flag; it measured 2.0e-2 rel err -
exactly at the harness gate - so fp16 scores ship instead (5e-3, 4x margin).
"""

import numpy as np
from contextlib import ExitStack

import concourse.bass as bass
import concourse.tile as tile
from concourse import bacc, mybir
from concourse.bass_utils import run_bass_kernel_spmd

F32 = mybir.dt.float32
F16 = mybir.dt.float16
F8 = mybir.dt.float8e4
I16 = mybir.dt.int16
AF = mybir.ActivationFunctionType
DR = mybir.MatmulPerfMode.DoubleRow
mul_op = mybir.AluOpType.mult
add_op = mybir.AluOpType.add

B, HH, WW, C = 4, 48, 48, 256
N = HH * WW            # 2304 keys per batch
NQ = N // 2            # 1152 queries per core
NH, HD, D2 = 8, 32, 16
NT = N // 128          # 18 key tiles
ROPE_BASE = 10000.0
SCALE = HD ** -0.5

# Schraudolph fp16 exp constants: i16 = round(1477.3195*x + B); the -44.7
# centers the piecewise-linear 2^frac sawtooth, +0.5 compensates truncation.
SCH_A = 1477.3195
SCH_B = 15360.0 - 44.7 + 0.5

QCH = [(0, 512), (1024, 128), (512, 512)]   # small chunk mid for overlap
KCH = [(0, 512), (512, 512), (1024, 512), (1536, 512), (2048, 256)]

IN_SPECS = [
    ("ctq", [128, NQ], F16), ("stq", [128, NQ], F16),
    ("xT0", [128, N], F16), ("xT1", [128, N], F16),  # x.T ci-halves fp16
    ("xq0", [128, NQ], F16), ("xq1", [128, NQ], F16),  # query-half slices
    ("wq", [128, 2, C], F16), ("wqr", [128, 2, C], F16),  # [:, ci_half, co]
    ("wk", [128, 2, C], F16), ("wkr", [128, 2, C], F16),
    ("wv0", [128, C], F16), ("wv1", [128, C], F16),
    ("wo0", [128, C], F16), ("wo1", [128, C], F16),
    ("qb", [128, 2], F32), ("rqb", [128, 2], F32),   # [:, cg]
    ("vb", [1, C], F16), ("bob", [1, C], F16),
    ("ones", [128, 128], F16),
    ("ct", [128, N], F16), ("st", [128, N], F16),    # unscaled cos/sin rows
]


ROUTE_DVE_ON = True
EXP_SPLIT = "route"  # "route" (per-tile engine routing) or "column"
FP8_SCORES = False  # fp8 scores rejected: k-side fp8 noise ~1.7e-2 rel vs 2e-2 gate


def route_dve(h, tp):
    # ~40% of exp tiles to DVE, interleaved so both engines stay fed.
    return ROUTE_DVE_ON and ((tp * 8 + h) * 2) % 5 < 2


def emit(tc, io, R=1):
    nc = tc.nc
    ctx = ExitStack()
    with ctx:
        consts = ctx.enter_context(tc.tile_pool(name="consts", bufs=1))
        sb = ctx.enter_context(tc.tile_pool(name="sb", bufs=1))
        tmp = ctx.enter_context(tc.tile_pool(name="tmp", bufs=4))
        ptpool = ctx.enter_context(tc.tile_pool(name="pt", bufs=10))
        otpool = ctx.enter_context(tc.tile_pool(name="oT", bufs=3))
        rpool = ctx.enter_context(tc.tile_pool(name="recip", bufs=2))
        scp = ctx.enter_context(tc.tile_pool(name="scp", bufs=6, space="PSUM"))
        avp = ctx.enter_context(tc.tile_pool(name="avp", bufs=1, space="PSUM"))
        # scp: 1-bank [128, 512] f32 tiles; 6 slots + av (2 banks) = 8 banks

        dtypes = {name: dt for name, _, dt in IN_SPECS}

        def load(name):
            shape = next(s for n, s, _ in IN_SPECS if n == name)
            t = consts.tile(shape, dtypes[name], tag=name)
            nc.sync.dma_start(t[:], io[name][:])
            return t

        # load order = first-use order so compute starts ASAP
        wq, wqr = load("wq"), load("wqr")
        xq = [load("xq0"), load("xq1")]
        ctq, stq = load("ctq"), load("stq")
        qb, rqb = load("qb"), load("rqb")
        wk, wkr = load("wk"), load("wkr")
        xT = [load("xT0"), load("xT1")]
        ct, st = load("ct"), load("st")
        wv = [load("wv0"), load("wv1")]
        vb, bob = load("vb"), load("bob")
        ones = load("ones")
        wo = [load("wo0"), load("wo1")]

        # persistent attention operand tiles
        if FP8_SCORES:
            qT8 = [sb.tile([128, 2, NQ], F8, tag=f"qT8{g}", name=f"qT8{g}")
                   for g in range(2)]
            kT8 = [sb.tile([128, 2, N], F8, tag=f"kT8{g}", name=f"kT8{g}")
                   for g in range(2)]
        else:
            qT8 = [sb.tile([128, NQ], F16, tag=f"qT16{g}", name=f"qT16{g}")
                   for g in range(2)]
            kT8 = [sb.tile([128, N], F16, tag=f"kT16{g}", name=f"kT16{g}")
                   for g in range(2)]
        vsb = sb.tile([128, NT * NH * 33], F16, tag="vsb")

        nc.gpsimd.memset(vsb[:], 1.0)

        if R > 1:
            loop_ctx = tc.For_i(0, R, 1)
            loop_ctx.__enter__()

        # ---- phase 1: projections + RoPE ---------------------------------
        def proj_rope(dst, w, wr, bias, rbias, xa, xb, cts, sts, chunks,
                      mode="q"):
            # dst[cg][:, 0, off:off+cw] = (W.T x + b)*cos + (Wr.T x + rb)*sin
            for off, cw in chunks:
                for cg in range(2):
                    ps0 = scp.tile([128, 512], F32, tag="sc")
                    nc.tensor.matmul(ps0[:, :cw],
                                     w[:, 0, bass.ts(cg, 128)],
                                     xa[:, off:off + cw],
                                     start=True, stop=False)
                    nc.tensor.matmul(ps0[:, :cw],
                                     w[:, 1, bass.ts(cg, 128)],
                                     xb[:, off:off + cw],
                                     start=False, stop=True)
                    ps1 = scp.tile([128, 512], F32, tag="sc")
                    nc.tensor.matmul(ps1[:, :cw],
                                     wr[:, 0, bass.ts(cg, 128)],
                                     xa[:, off:off + cw],
                                     start=True, stop=False)
                    nc.tensor.matmul(ps1[:, :cw],
                                     wr[:, 1, bass.ts(cg, 128)],
                                     xb[:, off:off + cw],
                                     start=False, stop=True)
                    t1 = tmp.tile([128, 512], F16, tag="t1")
                    b0 = bias[:, cg:cg + 1] if bias is not None else 0.0
                    b1 = rbias[:, cg:cg + 1] if rbias is not None else 0.0
                    nc.vector.scalar_tensor_tensor(
                        t1[:, 0:cw], ps0[:, 0:cw], b0,
                        cts[:, off:off + cw], op0=add_op, op1=mul_op)
                    t2 = tmp.tile([128, 512], F16, tag="t2")
                    nc.vector.scalar_tensor_tensor(
                        t2[:, 0:cw], ps1[:, 0:cw], b1,
                        sts[:, off:off + cw], op0=add_op, op1=mul_op)
                    if not FP8_SCORES:
                        # SBUF-only add runs on the otherwise-idle Pool
                        # engine, shortening the DVE projection chain.
                        nc.gpsimd.tensor_add(dst[cg][:, off:off + cw],
                                             t1[:, 0:cw], t2[:, 0:cw])
                    elif mode == "k":
                        # slot0 = k8, slot1 = k8/16 (pairs with 16*dq)
                        nc.vector.tensor_add(dst[cg][:, 0, off:off + cw],
                                             t1[:, 0:cw], t2[:, 0:cw])
                        nc.scalar.mul(dst[cg][:, 1, off:off + cw],
                                      dst[cg][:, 0, off:off + cw], 1.0 / 16.0)
                    else:
                        # ct/st pre-scaled x16: q16s = 16*q_rope;
                        # slot0 = q8 = q16s/16; slot1 = q16s - 16*q8 = 16*dq
                        q16s = tmp.tile([128, 512], F16, tag="q16s")
                        nc.vector.tensor_add(q16s[:, 0:cw],
                                             t1[:, 0:cw], t2[:, 0:cw])
                        nc.scalar.mul(dst[cg][:, 0, off:off + cw],
                                      q16s[:, 0:cw], 1.0 / 16.0)
                        nc.vector.scalar_tensor_tensor(
                            dst[cg][:, 1, off:off + cw],
                            dst[cg][:, 0, off:off + cw], -16.0,
                            q16s[:, 0:cw], op0=mul_op, op1=add_op)

        proj_rope(qT8, wq, wqr, qb, rqb, xq[0], xq[1], ctq, stq, QCH,
                  mode="q")
        proj_rope(kT8, wk, wkr, None, None, xT[0], xT[1], ct, st, KCH,
                  mode="k")

        # V projection fp16 + bias via K=1 ones matmul; copy into 33-wide
        # blocks (33rd col stays 1.0 from the memset) on ScalarE.
        for t in range(NT):
            ps = scp.tile([128, 512], F32, tag="sc")
            nc.tensor.matmul(ps[:, :C], xT[0][:, bass.ts(t, 128)], wv[0][:],
                             start=True, stop=False)
            nc.tensor.matmul(ps[:, :C], xT[1][:, bass.ts(t, 128)], wv[1][:],
                             start=False, stop=False)
            nc.tensor.matmul(ps[:, :C], ones[0:1, :], vb[:],
                             start=False, stop=True)
            vdst = vsb[:, t * NH * 33:(t + 1) * NH * 33]
            vdst = vdst.rearrange("p (h c) -> p h c", c=33)
            psrc = ps[:, 0:C].rearrange("p (h c) -> p h c", c=32)
            nc.scalar.copy(vdst[:, :, 0:32], psrc[:])


        if "dbg_q" in io:
            for g in range(2):
                qs = qT8[g][:, 0, :] if FP8_SCORES else qT8[g][:]
                ks = kT8[g][:, 0, :] if FP8_SCORES else kT8[g][:]
                nc.sync.dma_start(io["dbg_q"][g], qs)
                nc.sync.dma_start(io["dbg_k"][g], ks)

        # ---- phase 2: attention + output projection ----------------------
        # Head halves (hh) so av needs only 2 banks and double-buffers:
        # the normalize chain of one half overlaps the next half's matmuls.
        # Each tile's exp is column-split across ScalarE (native Exp) and
        # DVE (Schraudolph) so both engines work the same tile in parallel.
        for off, cw in QCH:
            c1 = (cw * 37) // 64 // 4 * 4       # ~58% of columns to ScalarE
            oT = [otpool.tile([128, 512], F16, tag=f"o{k}", name=f"o{k}")
                  for k in range(2)]
            for hh in range(2):
                av = avp.tile([128, 2, 512], F32, tag="av")
                for tp in range(NT // 2):
                    for hl in range(4):
                        h = 4 * hh + hl
                        cg, h4 = h // 4, h % 4
                        g, j = hl // 2, hl % 2
                        for i in range(2):
                            t = 2 * tp + i
                            sc = scp.tile([128, 512], F32, tag="sc")
                            if FP8_SCORES:
                                nc.tensor.matmul(
                                    sc[:, :cw],
                                    kT8[cg][bass.ts(h4, 32), :,
                                            bass.ts(t, 128)],
                                    qT8[cg][bass.ts(h4, 32), :, off:off + cw],
                                    start=True, stop=True, perf_mode=DR,
                                    tile_position=(32 * h4, 0))
                            else:
                                nc.tensor.matmul(
                                    sc[:, :cw],
                                    kT8[cg][bass.ts(h4, 32), bass.ts(t, 128)],
                                    qT8[cg][bass.ts(h4, 32), off:off + cw],
                                    start=True, stop=True,
                                    tile_position=(32 * h4, 0))
                            pt = ptpool.tile([128, 512], F16, tag="pt")
                            if (((tp * 8 + h) * 2 + i) * 2) % 5 < 2:
                                nc.vector.tensor_scalar(
                                    pt.bitcast(I16)[:, 0:cw], sc[:, 0:cw],
                                    SCH_A * SCALE, SCH_B, mul_op, add_op)
                            else:
                                nc.scalar.activation(pt[:, 0:cw], sc[:, 0:cw],
                                                     AF.Exp, scale=SCALE)
                            nc.tensor.matmul(
                                av[64 * j:64 * j + 33, g, 0:cw],
                                vsb[:, (t * NH + h) * 33:
                                       (t * NH + h + 1) * 33],
                                pt[:, 0:cw],
                                start=(t == 0), stop=(t == NT - 1),
                                tile_position=(0, 64 * j),
                                skip_group_check=True)

                # normalize this head half: reciprocal of denominator rows,
                # K=1 broadcast to 32 partitions, multiply into oT fp16.
                rsb = rpool.tile([128, 2, 512], F16, tag="rs")
                with nc.allow_low_precision("fp16 softmax scale rows"):
                    for j in range(2):
                        r = 64 * j + 32
                        nc.vector.reciprocal(rsb[r:r + 1, :, 0:cw],
                                             av[r:r + 1, :, 0:cw])
                rfsb = rpool.tile([128, 2, 512], F16, tag="rfsb")
                for g in range(2):
                    rf = scp.tile([128, 512], F32, tag="sc")
                    for j in range(2):
                        r = 64 * j + 32
                        nc.tensor.matmul(
                            rf[64 * j:64 * j + 32, 0:cw],
                            ones[r:r + 1, 0:32], rsb[r:r + 1, g, 0:cw],
                            start=True, stop=True,
                            tile_position=(r, 64 * j), skip_group_check=True)
                    nc.scalar.copy(rfsb[:, g, 0:cw], rf[:, 0:cw])
                for hl in range(4):
                    g, j = hl // 2, hl % 2
                    nc.vector.tensor_mul(
                        oT[hh][32 * hl:32 * hl + 32, 0:cw],
                        av[64 * j:64 * j + 32, g, 0:cw],
                        rfsb[64 * j:64 * j + 32, g, 0:cw])

            # output projection + bias, y DMA straight from PSUM
            for s in range(cw // 128):
                yps = scp.tile([128, 512], F32, tag="sc")
                nc.tensor.matmul(yps[:, :C], oT[0][:, bass.ts(s, 128)],
                                 wo[0][:], start=True, stop=False)
                nc.tensor.matmul(yps[:, :C], oT[1][:, bass.ts(s, 128)],
                                 wo[1][:], start=False, stop=False)
                nc.tensor.matmul(yps[:, :C], ones[0:1, 0:128], bob[:],
                                 start=False, stop=True)
                ysb = tmp.tile([128, 512], F32, tag="ysb")
                nc.scalar.copy(ysb[:, 0:C], yps[:, 0:C])
                nc.sync.dma_start(
                    io["y"][off + 128 * s: off + 128 * (s + 1), :],
                    ysb[:, 0:C])

        if R > 1:
            loop_ctx.__exit__(None, None, None)


def build_nc(R=1):
    nc = bacc.Bacc("TRN2", target_bir_lowering=False, debug=False,
                   enable_asserts=True, num_devices=8)
    io = {}
    for name, shape, dt in IN_SPECS:
        io[name] = nc.dram_tensor(name, shape, dt, kind="ExternalInput").ap()
    io["y"] = nc.dram_tensor("y", [NQ, C], F32, kind="ExternalOutput").ap()

    with tile.TileContext(nc) as tc:
        emit(tc, io, R=R)
    nc.compile()
    return nc


def host_inputs(x, Wq, q_bias, Wk, Wv, v_bias, Wo, bo):
    """Per-core input maps (host-side sharding + layout prep)."""
    import ml_dtypes
    f8d = ml_dtypes.float8_e4m3fn

    xf = np.ascontiguousarray(x.reshape(B, N, C))

    inv_freq = 1.0 / (ROPE_BASE ** (np.arange(0, HD, 2, dtype=np.float64) / HD))
    pos = np.arange(N, dtype=np.float64)
    ang = pos[:, None] * inv_freq[None, :]          # [N, 16]
    cos_t, sin_t = np.cos(ang), np.sin(ang)
    f = np.arange(128) % D2                         # row r -> freq index
    CT = cos_t[:, f].T                              # [128, N]
    ST = sin_t[:, f].T

    # signed rotate-half permutation RM [C, C]: partner = RM @ q
    RM = np.zeros((C, C), dtype=np.float64)
    for p in range(C):
        j = p % HD
        if j < D2:
            RM[p, p + D2] = -1.0
        else:
            RM[p, p - D2] = 1.0

    Wq64, Wk64 = Wq.astype(np.float64), Wk.astype(np.float64)
    Wqr = RM @ Wq64
    Wkr = RM @ Wk64
    rqb = RM @ q_bias.astype(np.float64)

    f16 = lambda a: np.ascontiguousarray(a, dtype=np.float16)
    f32 = lambda a: np.ascontiguousarray(a, dtype=np.float32)

    def wpack(WT):  # WT [C, C] -> [128, 2(ci half), C] fp16
        return f16(WT.reshape(2, 128, C).transpose(1, 0, 2))

    common = {
        "wq": wpack(Wq64.T), "wqr": wpack(Wqr.T),
        "wk": wpack(Wk64.T), "wkr": wpack(Wkr.T),
        "wv0": f16(Wv.T[0:128]), "wv1": f16(Wv.T[128:256]),
        "wo0": f16(Wo.T[0:128]), "wo1": f16(Wo.T[128:256]),
        "qb": f32(q_bias.reshape(2, 128).T),
        "rqb": f32(rqb.reshape(2, 128).T),
        "vb": f16(v_bias[None, :]), "bob": f16(bo[None, :]),
        "ones": np.ones((128, 128), dtype=np.float16),
        "ct": f16(CT), "st": f16(ST),
    }
    qsc = 16.0 if FP8_SCORES else 1.0   # q-residual trick pre-scale
    in_maps = []
    for core in range(8):
        b, qhalf = core // 2, core % 2
        qoff = qhalf * NQ
        xT = xf[b].T                                # [C, N]
        m = dict(common)
        m["ctq"] = f16(CT[:, qoff:qoff + NQ] * qsc)
        m["stq"] = f16(ST[:, qoff:qoff + NQ] * qsc)
        m["xT0"] = f16(xT[0:128])
        m["xT1"] = f16(xT[128:256])
        m["xq0"] = f16(xT[0:128, qoff:qoff + NQ])
        m["xq1"] = f16(xT[128:256, qoff:qoff + NQ])
        in_maps.append(m)
    return in_maps


_NC_CACHE = {}


def get_nc(R=1):
    if R not in _NC_CACHE:
        _NC_CACHE[R] = build_nc(R)
    return _NC_CACHE[R]


def kernel(**inputs):
    inputs = {k: np.asarray(v, dtype=np.float32) for k, v in inputs.items()}
    in_maps = host_inputs(**inputs)
    nc = get_nc()
    res = run_bass_kernel_spmd(nc, in_maps, core_ids=list(range(8)))
    out = np.empty((B, N, C), dtype=np.float32)
    for core in range(8):
        b, qhalf = core // 2, core % 2
        qoff = qhalf * NQ
        out[b, qoff:qoff + NQ, :] = res.results[core]["y"]
    return out.reshape(B, HH, WW, C)
